# revision 1
# baseline (speedup 1.0000x reference)
"""Distributed Bass kernel for GQA causal attention (B=2, S=2048, H=2048,
NH=16, NKV=4, HD=128) on 8 TRN2 NeuronCores.

Sharding: core c (0..7) handles batch b = c//4 and kv-group g = c%4
(4 query heads + 1 kv head, GQA groups kept intact).  wq/wk/wv are
column-sharded, wo row-sharded; each core emits a partial output
[H, S] (transposed) and the host sums the 4 group-partials per batch.

Layout strategy on device (f32 data, matmuls in float32r for full-rate
PE streaming; f32r provenance satisfied by declaring matmul-feeding
DRAM params / SBUF tiles as float32r and letting DVE/ACT output-cast):
  - x is fed pre-transposed (xT[h, s]) so QKV projections produce
    Q^T/K^T/V^T in [d, s] layout directly (d=128 = one partition tile).
  - RoPE rotate_half is a constant 128x128 matmul (R^T as lhsT);
    cos/sin are fed pre-transposed.
  - scores are computed transposed: ST[kj, qi] = K^T(kj)·Q(qi), so
    softmax needs no on-chip transposes. exp via ACT (scale folded),
    no max-subtraction (scores are O(1), f32 exp cannot overflow).
  - causal mask = additive -1e30 on the PSUM scores of the diagonal
    band only; fully-masked tiles are never computed.
  - O^T[d, qi] = V[kj, d]-as-lhsT @ P[kj, qi]; row-sums via ones-lhsT
    matmul; normalization via PE outer-product broadcast of 1/rowsum.
  - wo row-shard applied as lhsT to O^T giving out^T[e, s] partials.
"""

import math
import os
import sys

import ml_dtypes
import numpy as np

sys.path.insert(0, "/opt/trn_rl_repo")

import concourse.bass as bass
import concourse.mybir as mybir
import concourse.tile as tile
from concourse.bass_utils import run_bass_kernel_spmd

if os.environ.get("BASS_LDW_OPT", "") == "1":
    # experiment: let walrus dedupe consecutive identical LDWEIGHTS
    import concourse.bass_utils as _bu
    _orig_run_command = _bu.run_command

    def _patched_run_command(cmd, *a, **kw):
        cmd = [c.replace("--enable-ldw-opt=false", "--enable-ldw-opt=true")
               for c in cmd]
        return _orig_run_command(cmd, *a, **kw)

    _bu.run_command = _patched_run_command

B, S, H = 2, 2048, 2048
NH, NKV, HD = 16, 4, 128
NREP = NH // NKV
NCORES = 8
GH = 4                # q-heads per core (one kv group)
P = 128
SB = 512              # s-block width (matmul moving free dim)
NB = S // SB          # 4 s-blocks
NT = S // P           # 16 partition tiles along s / h / e
SCALE = 1.0 / math.sqrt(HD)
F32 = mybir.dt.float32
F32R = mybir.dt.float32r
BF16 = mybir.dt.bfloat16
MM_MODE = os.environ.get("BASS_MM_DTYPE", "f32r")  # bf16 | f32r | f32
USE_F32R = MM_MODE == "f32r"
MMDT = {"bf16": BF16, "f32r": F32R, "f32": F32}[MM_MODE]
NPMM = ml_dtypes.bfloat16 if MM_MODE == "bf16" else np.float32


def _consts():
    npdt = NPMM
    # rotate_half as matmul: rot = RT.T @ q  (RT is the lhsT)
    RT = np.zeros((P, P), npdt)
    idx = np.arange(64)
    RT[idx + 64, idx] = -1.0
    RT[idx, idx + 64] = 1.0
    # canonical causal additive triangle: 0 iff kj_local <= qi_local
    kjl = np.arange(P)[:, None]
    qil = np.arange(P)[None, :]
    masks = np.where(kjl <= qil, 0.0, -1e30).astype(np.float32)
    ident = np.eye(P, dtype=npdt)
    ones_k = np.ones((P, 1), npdt)
    ones_1 = np.ones((1, P), npdt)
    return RT, masks, ident, ones_k, ones_1


def build_nc():
    nc = bass.Bass()

    xT_d = nc.declare_dram_parameter("xT", [H, S], MMDT, isOutput=False)
    wq_d = nc.declare_dram_parameter("wq", [H, GH * HD], MMDT, isOutput=False)
    wk_d = nc.declare_dram_parameter("wk", [H, HD], MMDT, isOutput=False)
    wv_d = nc.declare_dram_parameter("wv", [H, HD], MMDT, isOutput=False)
    wo_d = nc.declare_dram_parameter("wo", [GH * HD, H], MMDT, isOutput=False)
    cosT_d = nc.declare_dram_parameter("cosT", [HD, S], F32, isOutput=False)
    sinT_d = nc.declare_dram_parameter("sinT", [HD, S], F32, isOutput=False)
    out_d = nc.declare_dram_parameter("out", [H, S], F32, isOutput=True)

    RT_np, masks_np, ident_np, ones_k_np, ones_1_np = _consts()
    RT_d = nc.inline_tensor(RT_np, "rot_t")
    masks_d = nc.inline_tensor(masks_np, "masks")
    ident_d = nc.inline_tensor(ident_np, "ident")
    ones_k_d = nc.inline_tensor(ones_k_np, "ones_k")
    ones_1_d = nc.inline_tensor(ones_1_np, "ones_1")

    def _mr(ap):
        """matmul-feeding const: reinterpret f32-typed DRAM as f32r only
        in f32r mode; bf16 consts are created in bf16 directly."""
        return ap.bitcast(F32R) if USE_F32R else ap

    with tile.TileContext(nc) as tc, \
         tc.tile_pool(name="persist", bufs=1) as persist:
        # constants (tiles only; DMAs issued AFTER the phase-1-critical
        # weight/x DMAs -- queue waits are cumulative in issue order)
        rt_sb = persist.tile([P, P], MMDT, tag="rt")
        masks_sb = persist.tile([P, P], F32, tag="masks")
        ident_sb = persist.tile([P, P], MMDT, tag="ident")
        ones_k_sb = persist.tile([P, 1], MMDT, tag="ones_k")
        ones_1_sb = persist.tile([1, P], MMDT, tag="ones_1")
        cos_sb = persist.tile([P, S], F32, tag="cos")
        sin_sb = persist.tile([P, S], F32, tag="sin")

        # resident weights (each element used once per s-block)
        wq_sb = persist.tile([P, NT, GH * HD], MMDT, tag="wq")  # 4 MB
        wk_sb = persist.tile([P, NT, HD], MMDT, tag="wk")       # 1 MB
        wv_sb = persist.tile([P, NT, HD], MMDT, tag="wv")       # 1 MB

        # per-head roped projections + V in [s, d] layout
        QR = [persist.tile([P, S], MMDT, tag=f"qr{h}", name=f"qr{h}")
              for h in range(GH)]
        KR = persist.tile([P, S], MMDT, tag="kr")
        VT = persist.tile([P, S], MMDT, tag="vt")  # V^T [d, s]
        VV = persist.tile([P, S], MMDT, tag="vv")  # V   [s, d] per kj tile

        # ---------------- Phase 1: projections + RoPE ----------------
        with (
            tc.tile_pool(name="xp", bufs=24) as xp,
            tc.tile_pool(name="p1w", bufs=3) as p1w,
            tc.tile_pool(name="p1ps", bufs=1, space="PSUM") as p1ps,
            tc.tile_pool(name="rotps", bufs=1, space="PSUM") as rotps,
        ):
            # critical-path DMAs first, interleaved per-t so the PE can
            # start the t-loop as soon as tile 0 lands
            xts0 = []
            for t in range(NT):
                xtt = xp.tile([P, SB], MMDT, tag="x", name=f"x0_{t}")
                nc.sync.dma_start(out=xtt, in_=xT_d[t * P:(t + 1) * P, 0:SB])
                nc.sync.dma_start(out=wq_sb[:, t, :],
                                  in_=wq_d[t * P:(t + 1) * P, :])
                nc.sync.dma_start(out=wk_sb[:, t, :],
                                  in_=wk_d[t * P:(t + 1) * P, :])
                nc.sync.dma_start(out=wv_sb[:, t, :],
                                  in_=wv_d[t * P:(t + 1) * P, :])
                xts0.append(xtt)
            xts1 = []
            for t in range(8):
                xtt = xp.tile([P, SB], MMDT, tag="x", name=f"x1_{t}")
                nc.sync.dma_start(out=xtt, in_=xT_d[t * P:(t + 1) * P,
                                                    SB:2 * SB])
                xts1.append(xtt)
            nc.sync.dma_start(out=rt_sb, in_=_mr(RT_d[:]))
            nc.sync.dma_start(out=ident_sb, in_=_mr(ident_d[:]))
            nc.sync.dma_start(out=ones_k_sb, in_=_mr(ones_k_d[:]))
            nc.sync.dma_start(out=ones_1_sb, in_=_mr(ones_1_d[:]))
            nc.sync.dma_start(out=cos_sb, in_=cosT_d[:])
            nc.sync.dma_start(out=sin_sb, in_=sinT_d[:])
            nc.sync.dma_start(out=masks_sb, in_=masks_d[:])
            for sb in range(NB):
                ssl = slice(sb * SB, (sb + 1) * SB)
                if sb == 0:
                    xt = xts0
                else:
                    xt = list(xts1) if sb == 1 else []
                    for t in range(len(xt), NT):
                        xtt = xp.tile([P, SB], MMDT, tag="x",
                                      name=f"x{sb}_{t}")
                        nc.sync.dma_start(out=xtt,
                                          in_=xT_d[t * P:(t + 1) * P, ssl])
                        xt.append(xtt)
                ps = [p1ps.tile([P, SB], F32, tag=f"ps{i}", name=f"ps{i}")
                      for i in range(6)]
                for t in range(NT):
                    st, sp = (t == 0), (t == NT - 1)
                    for h in range(GH):
                        nc.tensor.matmul(
                            ps[h], wq_sb[:, t, h * HD:(h + 1) * HD],
                            xt[t], start=st, stop=sp,
                        )
                    nc.tensor.matmul(ps[4], wk_sb[:, t, :], xt[t],
                                     start=st, stop=sp)
                    nc.tensor.matmul(ps[5], wv_sb[:, t, :], xt[t],
                                     start=st, stop=sp)
                # RoPE for Q heads and K; V^T plain copy.
                # DVE only ever reads f32/PSUM; f32r tiles are written by
                # output-cast (that's the "rounding" the verifier wants).
                for i in range(5):
                    raw_r = p1w.tile([P, SB], MMDT, tag="raw",
                                     name=f"raw{sb}_{i}")
                    nc.vector.tensor_copy(raw_r, ps[i])
                    rot = rotps.tile([P, SB], F32, tag="rot", name=f"rot{sb}_{i}")
                    nc.tensor.matmul(rot, rt_sb, raw_r)
                    t1 = p1w.tile([P, SB], F32, tag="t1", name=f"t1_{sb}_{i}")
                    nc.vector.tensor_mul(t1, raw_r, cos_sb[:, ssl])
                    t2 = p1w.tile([P, SB], F32, tag="t2", name=f"t2_{sb}_{i}")
                    nc.vector.tensor_mul(t2, rot, sin_sb[:, ssl])
                    dst = QR[i] if i < GH else KR
                    nc.vector.tensor_add(dst[:, ssl], t1, t2)
                nc.vector.tensor_copy(VT[:, ssl], ps[5])
                # transpose this block's V^T -> V[s, d] tiles right away
                for tt in range(SB // P):
                    t = sb * (SB // P) + tt
                    vps = rotps.tile([P, P], MMDT, tag="vtr", name=f"vtr{t}")
                    nc.tensor.transpose(vps, VT[:, t * P:(t + 1) * P],
                                        ident_sb)
                    nc.vector.tensor_copy(VV[:, t * P:(t + 1) * P], vps)

        # ---------------- Phase 2: attention (h outer) ----------------
        with (
            tc.tile_pool(name="attnp", bufs=1) as attnp,
            tc.tile_pool(name="p2w", bufs=8) as p2w,
            tc.tile_pool(name="p3w", bufs=4) as p3w,
            tc.tile_pool(name="mixps", bufs=6, space="PSUM") as mixps,
            tc.tile_pool(name="otps", bufs=2, space="PSUM") as otps,
        ):
            OT = [attnp.tile([P, S], MMDT, tag=f"ot{h}", name=f"ot{h}")
                  for h in range(GH)]
            # wo shares wq_sb's slot (dead after phase 1); prefetch during
            # attention so phase 3 starts without a DMA stall
            wo_sb = wq_sb.rearrange("p a b -> p (a b)").rearrange(
                "p (g e) -> p g e", g=GH)
            for hh in range(GH):
                nc.sync.dma_start(out=wo_sb[:, hh, :],
                                  in_=wo_d[hh * P:(hh + 1) * P, :])

            pending_norm = None
            for h in range(GH):
                for qb in range(NB):
                    qsl = slice(qb * SB, (qb + 1) * SB)
                    nkj = 4 * (qb + 1)
                    ot_ps = otps.tile([P, SB], F32, tag="ot",
                                      name=f"otp{h}_{qb}")
                    rs_ps = mixps.tile([1, SB], F32, tag="mix",
                                       name=f"rsp{h}_{qb}")
                    for kj in range(nkj):
                        j = kj - (nkj - 4)
                        q0 = 0 if j < 0 else P * j
                        W = SB - q0
                        qslw = slice(qb * SB + q0, (qb + 1) * SB)
                        st_ps = mixps.tile([P, SB], F32, tag="mix",
                                           name=f"st{h}_{qb}_{kj}")
                        nc.tensor.matmul(st_ps[:, :W],
                                         KR[:, kj * P:(kj + 1) * P],
                                         QR[h][:, qslw])
                        if j >= 0:
                            nc.vector.tensor_add(st_ps[:, :P],
                                                 st_ps[:, :P], masks_sb)
                        p_sb = p2w.tile([P, SB], MMDT, tag="p",
                                        name=f"p{h}_{qb}_{kj}")
                        nc.scalar.activation(
                            p_sb[:, :W], st_ps[:, :W],
                            mybir.ActivationFunctionType.Exp, scale=SCALE)
                        first, last = (kj == 0), (kj == nkj - 1)
                        nc.tensor.matmul(ot_ps[:, q0:],
                                         VV[:, kj * P:(kj + 1) * P],
                                         p_sb[:, :W],
                                         start=first, stop=last,
                                         skip_group_check=True)
                        nc.tensor.matmul(rs_ps[:, q0:], ones_k_sb,
                                         p_sb[:, :W],
                                         start=first, stop=last,
                                         skip_group_check=True)
                    last_block = (h == GH - 1 and qb == NB - 1)
                    if not last_block:
                        # 1-lane DVE recip: slow, but OFF the PE critical
                        # path; broadcast + normalize run during the NEXT
                        # block.
                        rec = p2w.tile([1, SB], MMDT, tag="rec",
                                       name=f"rec{h}_{qb}", bufs=2)
                        with nc.allow_low_precision(
                                reason="4-byte f32r storage of 1/rowsum"):
                            nc.vector.reciprocal(rec, rs_ps)

                        def _norm(h=h, qb=qb, qsl=qsl, rec=rec,
                                  ot_ps=ot_ps):
                            bc_ps = mixps.tile([P, SB], F32, tag="mix",
                                               name=f"bcp{h}_{qb}")
                            nc.tensor.matmul(bc_ps, ones_1_sb, rec)
                            bc_sb = p2w.tile([P, SB], F32, tag="bcs",
                                             name=f"bcs{h}_{qb}", bufs=2)
                            nc.vector.tensor_copy(bc_sb, bc_ps)
                            nc.vector.tensor_mul(OT[h][:, qsl], ot_ps,
                                                 bc_sb)
                    else:
                        # final block is tail-latency-critical: stage the
                        # rowsum via ACT and do a 128-lane reciprocal
                        rs_sb = p2w.tile([1, SB], MMDT, tag="rec",
                                         name=f"recL", bufs=2)
                        nc.scalar.copy(rs_sb, rs_ps)

                        def _norm(h=h, qb=qb, qsl=qsl, rs_sb=rs_sb,
                                  ot_ps=ot_ps):
                            bc_ps = mixps.tile([P, SB], F32, tag="mix",
                                               name=f"bcpL")
                            nc.tensor.matmul(bc_ps, ones_1_sb, rs_sb)
                            bc_sb = p2w.tile([P, SB], F32, tag="bcs",
                                             name=f"bcsL", bufs=2)
                            nc.vector.reciprocal(bc_sb, bc_ps)
                            nc.vector.tensor_mul(OT[h][:, qsl], ot_ps,
                                                 bc_sb)

                    if pending_norm is not None:
                        pending_norm()
                    pending_norm = _norm
            if pending_norm is not None:
                pending_norm()

            # ------------- Phase 3: output projection -------------
            for sb in range(NB):
                for e in range(NT):
                    ssl = slice(sb * SB, (sb + 1) * SB)
                    o_ps = mixps.tile([P, SB], F32, tag="mix",
                                      name=f"wops{e}_{sb}")
                    for hh in range(GH):
                        nc.tensor.matmul(
                            o_ps, wo_sb[:, hh, e * P:(e + 1) * P],
                            OT[hh][:, ssl],
                            start=(hh == 0), stop=(hh == GH - 1))
                    oe = p3w.tile([P, SB], F32, tag="oe",
                                  name=f"oe{e}_{sb}")
                    nc.vector.tensor_copy(oe, o_ps)
                    nc.sync.dma_start(
                        out=out_d[e * P:(e + 1) * P, ssl], in_=oe)

    _hoist_matmul_waits(nc)
    return nc


_HOIST_OPS = {"Matmult", "DMACopy"}


def _hoist_matmul_waits(nc):
    """Self-loading f32r matmuls (and direct2d DMAs) only support ONE
    sync-wait — walrus puts all waits on one ISA struct.  Hoist extra
    waits onto standalone single-wait EventSemaphores inserted right
    before the offending instruction on the same engine."""
    n_fixed = 0
    for fn in nc.m.functions:
        for blk in fn.blocks:
            out = []
            for inst in blk.instructions:
                si = inst.sync_info
                if (inst.opcode != "EventSemaphore" and si is not None
                        and si.on_wait is not None and len(si.on_wait) > 1):
                    waits = list(si.on_wait)
                    for wi, w in enumerate(waits[:-1]):
                        out.append(mybir.InstEventSemaphore(
                            name=f"hoistw_{inst.name}_{wi}", ins=[], outs=[],
                            sync_info=mybir.SyncInfo(on_wait=[w],
                                                     on_update=[]),
                            engine=inst.engine))
                    inst.sync_info = mybir.SyncInfo(
                        on_wait=[waits[-1]],
                        on_update=list(si.on_update or []))
                    n_fixed += 1
                out.append(inst)
            blk.instructions = out
    return n_fixed


def make_in_maps(x, cos, sin, wq, wk, wv, wo):
    cosT = np.ascontiguousarray(cos.T.astype(np.float32))
    sinT = np.ascontiguousarray(sin.T.astype(np.float32))
    xT = [np.ascontiguousarray(x[b].T.astype(NPMM)) for b in range(B)]
    wq, wk, wv, wo = (a.astype(NPMM) for a in (wq, wk, wv, wo))
    in_maps = []
    for c in range(NCORES):
        b, g = divmod(c, NKV)
        in_maps.append({
            "xT": xT[b],
            "wq": np.ascontiguousarray(wq[:, g * GH * HD:(g + 1) * GH * HD]),
            "wk": np.ascontiguousarray(wk[:, g * HD:(g + 1) * HD]),
            "wv": np.ascontiguousarray(wv[:, g * HD:(g + 1) * HD]),
            "wo": np.ascontiguousarray(wo[g * GH * HD:(g + 1) * GH * HD, :]),
            "cosT": cosT,
            "sinT": sinT,
        })
    return in_maps


_NC_CACHE = {}


def _get_nc():
    if "nc" not in _NC_CACHE:
        _NC_CACHE["nc"] = build_nc()
    return _NC_CACHE["nc"]


def run(x, cos, sin, wq, wk, wv, wo, **spmd_kwargs):
    nc = _get_nc()
    in_maps = make_in_maps(x, cos, sin, wq, wk, wv, wo)
    res = run_bass_kernel_spmd(nc, in_maps, core_ids=list(range(NCORES)),
                               **spmd_kwargs)
    outs = [np.asarray(res.results[c]["out"]) for c in range(NCORES)]
    full = np.empty((B, S, H), np.float32)
    for b in range(B):
        acc = outs[4 * b]
        for g in range(1, NKV):
            acc = acc + outs[4 * b + g]
        full[b] = acc.T
    return full, res


def kernel(**inputs):
    out, _ = run(**inputs)
    return out


if __name__ == "__main__":
    import tempfile
    from concourse.bass_utils import compile_bir_kernel

    nc = build_nc()
    print("graph built OK")
    if os.environ.get("COMPILE_CHECK", "1") == "1":
        td = tempfile.mkdtemp(prefix="bass_compile_")
        neff = compile_bir_kernel(nc.to_json_bytes(), td, "kernel.neff")
        print(f"compiled OK: {neff}")

